# revision 7
# baseline (speedup 1.0000x reference)
"""Multi-head self-attention (no mask) on 8 TRN2 NeuronCores.

Tensor-parallel over heads (2 heads/core) for QKV + attention; an
AllToAll per (batch, head) re-shards to row-parallel for the output
projection. All inputs host-precast to bf16 and pre-tiled to
[128, k, cols] so every load is a single wide DMA; v-bias folded into
bo' = bo + Wo @ bv (softmax rows sum to 1, so this is exact).

The PE is in-order, so the emission order IS the schedule. A
cycle-fraction weaver interleaves independent matmul streams at
single-matmul granularity so the PE never blocks on the ACT-engine
exp drain of the scores PSUM banks:

  A0: QKV projections for batch 0 (chunks 0-3), PE-saturated.
  W1: QKV batch 1 (chunks 4-7) woven with attention(b0).
      AllToAll(b0,h) issued as each head completes; its aT gather
      runs on the gpsimd DMA queue straight after.
  W2: attention(b1) woven with out-proj(b0) tiles. Each tile is
      emitted as two 8-matmul halves (head-0 sources then head-1
      sources) so the first half only needs the first b0 AllToAll.
  W3: remaining b0 tiles + out-proj(b1), h0-halves juggled 3-deep
      across the final AllToAll's latency.

Softmax (no max-subtraction; scores are O(5)): denominators via a DVE
pairwise tree over exp k-tiles + a ones^T matmul, then
reciprocal_approx_fast (f32) and a tiny broadcast matmul.
"""

import numpy as np

import concourse.bass as bass
import concourse.tile as tile
from concourse import bacc, mybir
from concourse.bass_utils import run_bass_kernel_spmd

F32 = mybir.dt.float32
BF16 = mybir.dt.bfloat16

B, S, H = 2, 2048, 2048
NH, HD = 16, 128
NC = 8
BS = B * S           # 4096 rows total
FL = H // NC         # 256 features per core (2 heads)
HL = NH // NC        # 2 local heads
RPB = S // NC        # 256 output rows per core per batch
K16 = H // 128       # 16 contraction tiles
CW = 512             # projection row-chunk width
NCHUNK = BS // CW    # 8
QC = 512             # attention q-chunk width
NQC = S // QC        # 4 q-chunks per (head, batch)
SCALE = 1.0 / float(np.sqrt(HD))

_CACHED = None


def _weave(streams):
    """streams: list of (iterator of (cycles, emit_fn), total_cycles).

    Emits one step at a time from the stream with the lowest emitted
    fraction, so concurrent work interleaves at matmul granularity.
    """
    state = []
    for spec in streams:
        it, total = spec[0], spec[1]
        acc0 = spec[2] if len(spec) > 2 else 0
        state.append({"it": iter(it), "total": max(total, 1), "acc": acc0,
                      "done": False, "pending": None})
    while True:
        best = None
        for st in state:
            if st["done"]:
                continue
            if st["pending"] is None:
                try:
                    st["pending"] = next(st["it"])
                except StopIteration:
                    st["done"] = True
                    continue
            if best is None or st["acc"] / st["total"] < best["acc"] / best["total"]:
                best = st
        if best is None:
            return
        cyc, fn = best["pending"]
        best["pending"] = None
        fn()
        best["acc"] += cyc


def _build():
    nc = bacc.Bacc("TRN2", target_bir_lowering=False, debug=False, num_devices=NC)

    # pre-tiled inputs: [128 partitions, k-tile, columns]
    xT_d = nc.dram_tensor("xT3", [128, K16, BS], BF16, kind="ExternalInput")
    wqT_d = nc.dram_tensor("wqT3", [128, K16, FL], BF16, kind="ExternalInput")
    wkT_d = nc.dram_tensor("wkT3", [128, K16, FL], BF16, kind="ExternalInput")
    wvT_d = nc.dram_tensor("wvT3", [128, K16, FL], BF16, kind="ExternalInput")
    bq_d = nc.dram_tensor("bq", [128, HL], F32, kind="ExternalInput")
    bk_d = nc.dram_tensor("bk", [128, HL], F32, kind="ExternalInput")
    woT_d = nc.dram_tensor("woT3", [128, K16, H], BF16, kind="ExternalInput")
    bo_d = nc.dram_tensor("bo_bc", [128, H], BF16, kind="ExternalInput")
    onesb_d = nc.dram_tensor("ones_bf", [128, 128], BF16, kind="ExternalInput")
    out_d = nc.dram_tensor("out", [B * RPB, H], F32, kind="ExternalOutput")

    with tile.TileContext(nc) as tc:
        with (
            tc.tile_pool(name="consts", bufs=1) as cstp,
            tc.tile_pool(name="dram", bufs=1, space="DRAM") as dp,
            tc.tile_pool(name="qkv", bufs=1) as qkvp,
            tc.tile_pool(name="expp", bufs=2) as ep,
            tc.tile_pool(name="tree", bufs=1) as trp,
            tc.tile_pool(name="attp", bufs=2) as ap_,
            tc.tile_pool(name="recp", bufs=2) as rp,
            tc.tile_pool(name="aTp", bufs=1) as atp,
            tc.tile_pool(name="outC", bufs=3) as ocp,
            tc.tile_pool(name="psum", bufs=1, space="PSUM") as pp,
        ):
            ones_bf = cstp.tile([128, 128], BF16)

            a2a_in = [
                [dp.tile([NC, 128, RPB], BF16, name=f"a2a_in{b}{h}") for h in range(HL)]
                for b in range(B)
            ]
            a2a_out = [
                [dp.tile([NC, 128, RPB], BF16, name=f"a2a_out{b}{h}") for h in range(HL)]
                for b in range(B)
            ]

            qT_sb = qkvp.tile([128, HL, BS], BF16)   # [hd, head, col(b,s)]
            kT_sb = qkvp.tile([128, HL, BS], BF16)
            v_sb = qkvp.tile([128, BS // 128, FL], BF16)  # [row%128, rowtile, feat]
            # gathered attention, per (batch, local head): [hd, src core, rows]
            aT = [
                [atp.tile([128, NC, RPB], BF16, name=f"aT{b}{h}") for h in range(HL)]
                for b in range(B)
            ]

            bchunks = [
                (b, h, qc)
                for b in range(B)
                for h in range(HL)
                for qc in range(NQC)
            ]

            # ---------- attention chunk blocks ----------
            def s_block(j):
                b, h, qc = bchunks[j]
                base = b * S
                hold = {}

                def step(km):
                    if km == 0:
                        hold["expT"] = ep.tile([128, K16, QC], BF16, tag="expT",
                                               name="expT")
                        hold["sums"] = pp.tile([128, QC], F32, tag="sums", bufs=1,
                                               name="sums")
                    pss = pp.tile([128, QC], F32, tag="pss", bufs=3)
                    nc.tensor.matmul(
                        pss[:],
                        kT_sb[:, h, base + km * 128: base + (km + 1) * 128],
                        qT_sb[:, h, base + qc * QC: base + (qc + 1) * QC],
                        start=True,
                        stop=True,
                    )
                    nc.scalar.activation(
                        hold["expT"][:, km, :],
                        pss[:],
                        mybir.ActivationFunctionType.Exp,
                        scale=SCALE,
                    )

                def epi():
                    expT = hold["expT"]
                    s1 = trp.tile([128, 8, QC], BF16, tag="s1")
                    nc.vector.tensor_add(s1[:], expT[:, 0:8, :], expT[:, 8:16, :])
                    s2 = trp.tile([128, 4, QC], BF16, tag="s2")
                    nc.vector.tensor_add(s2[:], s1[:, 0:4, :], s1[:, 4:8, :])
                    s3 = trp.tile([128, 2, QC], BF16, tag="s3")
                    nc.vector.tensor_add(s3[:], s2[:, 0:2, :], s2[:, 2:4, :])
                    s4 = trp.tile([128, QC], BF16, tag="s4", bufs=2)
                    nc.vector.tensor_add(s4[:], s3[:, 0, :], s3[:, 1, :])
                    nc.tensor.matmul(
                        hold["sums"][0:1, :], ones_bf[:, 0:1], s4[:],
                        start=True, stop=True,
                    )
                    recip = rp.tile([1, QC], F32, tag="recip")
                    nc.vector.reciprocal_approx_fast(
                        recip[0:1, :], hold["sums"][0:1, :]
                    )
                    recb = rp.tile([1, QC], BF16, tag="recb")
                    nc.vector.tensor_copy(recb[0:1, :], recip[0:1, :])
                    hold["recb"] = recb

                def gen():
                    for km in range(K16):
                        yield (512, lambda km=km: step(km))
                    yield (130, epi)

                return gen(), hold

            def p_block(j, hold):
                b, h, qc = bchunks[j]

                def step(km):
                    if km == 0:
                        nc.tensor.matmul(
                            hold["sums"][:],
                            ones_bf[0:1, :],
                            hold["recb"][0:1, :],
                            start=True,
                            stop=True,
                        )
                        hold["psa"] = pp.tile([128, QC], F32, tag="psa", bufs=2,
                                              name="psa")
                    nc.tensor.matmul(
                        hold["psa"][:],
                        v_sb[:, (S // 128) * b + km, h * 128:(h + 1) * 128],
                        hold["expT"][:, km, :],
                        start=(km == 0),
                        stop=(km == K16 - 1),
                    )

                def epi():
                    rb = rp.tile([128, QC], BF16, tag="rb")
                    nc.vector.tensor_copy(rb[:], hold["sums"][:])
                    att = ap_.tile([128, QC], BF16, tag="att")
                    nc.vector.tensor_mul(att[:], hold["psa"][:], rb[:])
                    for half in range(2):
                        dest = qc * 2 + half
                        nc.gpsimd.dma_start(
                            a2a_in[b][h][dest, :, :],
                            att[:, half * RPB:(half + 1) * RPB],
                        )
                    if qc == NQC - 1 and not (b == 0 and h == 1):
                        nc.gpsimd.collective_compute(
                            "AllToAll",
                            mybir.AluOpType.bypass,
                            ins=[a2a_in[b][h].opt()],
                            outs=[a2a_out[b][h].opt()],
                            replica_groups=[list(range(NC))],
                        )
                        # gather straight onto the gpsimd queue: one DMA
                        # per source core into the [hd, src, rows] tile
                        for src in range(NC):
                            nc.gpsimd.dma_start(
                                aT[b][h][:, src, :],
                                a2a_out[b][h][src, :, :],
                            )

                def gen():
                    for km in range(K16):
                        yield (512, lambda km=km: step(km))
                    yield (130, epi)

                return gen()

            def issue_a2a(b, h):
                nc.gpsimd.collective_compute(
                    "AllToAll",
                    mybir.AluOpType.bypass,
                    ins=[a2a_in[b][h].opt()],
                    outs=[a2a_out[b][h].opt()],
                    replica_groups=[list(range(NC))],
                )
                for src_ in range(NC):
                    nc.gpsimd.dma_start(
                        aT[b][h][:, src_, :],
                        a2a_out[b][h][src_, :, :],
                    )

            def b_stream(j0, j1):
                for j in range(j0, j1):
                    sgen, hold = s_block(j)
                    yield from sgen
                    yield from p_block(j, hold)

            # ---------- out-projection tiles (emitted as two halves) ----
            def c_tile_halves(b, n, m, won_t, bo_sb, tag, bufs, out_eng=None):
                """k-tile g = 2*src + hh; half hh uses aT[b][hh]."""
                ps = {}

                def epi():
                    ot = ocp.tile([128, 512], F32, tag="ot")
                    nc.vector.tensor_add(
                        ot[:], ps["t"][:], bo_sb[:, n * 512:(n + 1) * 512]
                    )
                    (out_eng or nc.sync).dma_start(
                        out_d.ap()[b * RPB + m * 128: b * RPB + (m + 1) * 128,
                                   n * 512:(n + 1) * 512],
                        ot[:],
                    )

                def half(hh):
                    def step(i):
                        if hh == 0 and i == 0:
                            ps["t"] = pp.tile([128, 512], F32, tag=tag, bufs=bufs,
                                              name="psc")
                        nc.tensor.matmul(
                            ps["t"][:],
                            aT[b][hh][:, i, m * 128:(m + 1) * 128],
                            won_t[:, 2 * i + hh, :],
                            start=(hh == 0 and i == 0),
                            stop=(hh == 1 and i == NC - 1),
                        )

                    for i in range(NC):
                        yield (512, lambda i=i: step(i))
                    if hh == 1:
                        yield (130, epi)

                return half

            # ---------- phase A (projections) ----------
            with (
                tc.tile_pool(name="wgt", bufs=1) as wp,
                tc.tile_pool(name="xbf", bufs=2) as xbp,
            ):
                wq_sb = wp.tile([128, K16, FL], BF16, tag="wq")
                nc.sync.dma_start(wq_sb[:, 0:4, :], wqT_d.ap()[:, 0:4, :])
                x_tiles = {}

                def load_x(c, split=1, eng=None):
                    if c >= NCHUNK:
                        return
                    t = xbp.tile([128, K16, CW], BF16, tag="xbf", name="xc")
                    kk = K16 // split
                    for si in range(split):
                        (eng or nc.sync).dma_start(
                            t[:, si * kk:(si + 1) * kk, :],
                            xT_d.ap()[:, si * kk:(si + 1) * kk,
                                      c * CW:(c + 1) * CW],
                        )
                    x_tiles[c] = t

                load_x(0, split=8, eng=nc.scalar)
                nc.sync.dma_start(wq_sb[:, 4:16, :], wqT_d.ap()[:, 4:16, :])
                wk_sb = wp.tile([128, K16, FL], BF16, tag="wk")
                nc.sync.dma_start(wk_sb[:], wkT_d.ap()[:])
                wv_sb = wp.tile([128, K16, FL], BF16, tag="wv")
                nc.sync.dma_start(wv_sb[:], wvT_d.ap()[:])
                load_x(1, split=2)
                nc.sync.dma_start(ones_bf[:], onesb_d.ap()[:])
                bq_sb = wp.tile([128, HL], F32, tag="bq")
                nc.sync.dma_start(bq_sb[:], bq_d.ap()[:])
                bk_sb = wp.tile([128, HL], F32, tag="bk")
                nc.sync.dma_start(bk_sb[:], bk_d.ap()[:])

                def a_chunk_stream(c):
                    """qm0, km0, qm1, km1 (512cy steps), v0-3 (256cy steps)."""
                    xc = x_tiles[c]
                    for m in range(HL):
                        for w_sb, b_sb, dst in (
                            (wq_sb, bq_sb, qT_sb),
                            (wk_sb, bk_sb, kT_sb),
                        ):
                            ps = {}

                            def step(k, m=m, w_sb=w_sb, ps=ps):
                                if k == 0:
                                    ps["t"] = pp.tile([128, CW], F32, tag="psq",
                                                      bufs=2, name="psqk")
                                nc.tensor.matmul(
                                    ps["t"][:],
                                    w_sb[:, k, m * 128:(m + 1) * 128],
                                    xc[:, k, :],
                                    start=(k == 0),
                                    stop=(k == K16 - 1),
                                )

                            def epi(m=m, b_sb=b_sb, dst=dst, ps=ps):
                                nc.vector.tensor_scalar_add(
                                    dst[:, m, c * CW:(c + 1) * CW],
                                    ps["t"][:],
                                    b_sb[:, m:m + 1],
                                )

                            for k in range(K16):
                                yield (512, lambda k=k, step=step: step(k))
                            yield (90, epi)
                    for m2 in range(CW // 128):
                        ps = {}

                        def stepv(k, m2=m2, ps=ps):
                            if k == 0:
                                ps["t"] = pp.tile([128, FL], F32, tag="psa",
                                                  bufs=2, name="psv")
                            nc.tensor.matmul(
                                ps["t"][:],
                                xc[:, k, m2 * 128:(m2 + 1) * 128],
                                wv_sb[:, k, :],
                                start=(k == 0),
                                stop=(k == K16 - 1),
                            )

                        def epiv(m2=m2, ps=ps):
                            i = c * (CW // 128) + m2
                            nc.vector.tensor_copy(v_sb[:, i, :], ps["t"][:])

                        for k in range(K16):
                            yield (256, lambda k=k, stepv=stepv: stepv(k))
                        yield (60, epiv)

                # A0: chunks 0-3 sequential (attention not ready yet)
                for c in range(4):
                    load_x(c + 2, split=2)
                    for cyc, fn in a_chunk_stream(c):
                        fn()

                # W1: A chunks 4-7 woven with attention(b0) chunks 0-7
                def a_rest():
                    for c in range(4, 8):
                        yield (0, lambda c=c: load_x(c + 2, split=2))
                        yield from a_chunk_stream(c)

                _weave([
                    (a_rest(), 4 * (4 * K16 * 512 + 4 * K16 * 256)),
                    (b_stream(0, 8), 8 * 2 * K16 * 512),
                ])
            # wgt/xbf pools closed: SBUF freed for Wo residency

            with tc.tile_pool(name="woP", bufs=1) as wop:
                issue_a2a(0, 1)
                bo_sb = wop.tile([128, H], BF16, tag="bo")
                nc.sync.dma_start(bo_sb[:], bo_d.ap()[:])
                won = []
                for n in range(4):
                    t = wop.tile([128, K16, 512], BF16, tag=f"won{n}")
                    nc.sync.dma_start(
                        t[:], woT_d.ap()[:, :, n * 512:(n + 1) * 512]
                    )
                    won.append(t)

                c0_tiles = [(n, m) for n in range(4) for m in range(2)]

                def halves_for(tiles, tag, bufs):
                    return {
                        (n, m, b): c_tile_halves(b, n, m, won[n], bo_sb, tag, bufs)
                        for (n, m, b) in tiles
                    }

                # W2: attention(b1) woven with 6 out-proj(b0) tiles,
                # h0-half before h1-half so the first b0 AllToAll suffices
                w2_tiles = [(n, m, 0) for (n, m) in c0_tiles[:6]]
                hv = halves_for(w2_tiles, "psq", 2)
                w2_order = []
                keys = [k for k in hv]
                for idx, key in enumerate(keys):
                    w2_order.append((key, 0))
                    if idx >= 1:
                        w2_order.append((keys[idx - 1], 1))
                w2_order.append((keys[-1], 1))
                # -> T0h0, T1h0, T0h1, T2h0, T1h1, ... T5h0, T4h1, T5h1

                def c_stream():
                    for key, hh in w2_order:
                        yield from hv[key](hh)

                _weave([
                    (b_stream(8, 16), 8 * 2 * K16 * 512),
                    (c_stream(), len(w2_tiles) * K16 * 512, 10 * 512),
                ])

                # W3: b1 h0-halves run 5 deep (psq 2 + pss 3 banks) and the
                # two leftover b0 tiles (psa banks) cover the final AllToAll
                # before any h1-half needs its gather.
                b1k = [(n, m) for (n, m) in c0_tiles]   # T8..T15
                tags3 = ["psq", "psq", "pss", "pss", "pss",
                         "psa", "psa", "psq"]
                bufs3 = {"psq": 2, "pss": 3, "psa": 2}
                hv3 = {}
                for (n, m), tg in zip(b1k, tags3):
                    hv3[(n, m, 1)] = c_tile_halves(
                        1, n, m, won[n], bo_sb, tg, bufs3[tg],
                        out_eng=nc.scalar)
                for (n, m) in c0_tiles[6:]:
                    hv3[(n, m, 0)] = c_tile_halves(
                        0, n, m, won[n], bo_sb, "psa", 2, out_eng=nc.scalar)
                bk1 = [(n, m, 1) for (n, m) in b1k]
                bk0 = [(n, m, 0) for (n, m) in c0_tiles[6:]]
                # cover: 2 full b0 tiles + 7 h0-halves (psq2+pss3+psa2)
                # before the first h1-half needs the last AllToAll's gather
                order3 = [(bk0[0], 0), (bk0[0], 1),
                          (bk0[1], 0), (bk0[1], 1),
                          (bk1[0], 0), (bk1[1], 0),      # psq
                          (bk1[2], 0), (bk1[3], 0), (bk1[4], 0),  # pss
                          (bk1[5], 0), (bk1[6], 0),      # psa
                          (bk1[0], 1), (bk1[7], 0),      # psq free -> T15h0
                          (bk1[1], 1), (bk1[2], 1), (bk1[3], 1),
                          (bk1[4], 1), (bk1[5], 1), (bk1[6], 1),
                          (bk1[7], 1)]
                for key, hh in order3:
                    for cyc, fn in hv3[key](hh):
                        fn()

    nc.compile()
    return nc


def _get_nc():
    global _CACHED
    if _CACHED is None:
        _CACHED = _build()
    return _CACHED


def _tile128(a):
    """[(16*128), cols] -> [128, 16, cols]"""
    k, cols = a.shape[0] // 128, a.shape[1]
    return np.ascontiguousarray(a.reshape(K16, 128, cols).transpose(1, 0, 2))


def _prep_in_maps(x, Wq, bq, Wk, bk, Wv, bv, Wo, bo):
    import ml_dtypes

    BF = ml_dtypes.bfloat16
    xT = _tile128(np.ascontiguousarray(x.reshape(BS, H).T).astype(BF))
    woT = _tile128(np.ascontiguousarray(Wo.T).astype(BF))
    bo2 = bo.astype(np.float64) + Wo.astype(np.float64) @ bv.astype(np.float64)
    bo_bc = np.ascontiguousarray(
        np.broadcast_to(bo2.astype(np.float32), (128, H))
    ).astype(BF)
    ones_bf = np.ones((128, 128), BF)
    in_maps = []
    for c in range(NC):
        sl = slice(FL * c, FL * (c + 1))
        in_maps.append(
            {
                "xT3": xT,
                "wqT3": _tile128(np.ascontiguousarray(Wq[sl, :].T).astype(BF)),
                "wkT3": _tile128(np.ascontiguousarray(Wk[sl, :].T).astype(BF)),
                "wvT3": _tile128(np.ascontiguousarray(Wv[sl, :].T).astype(BF)),
                "bq": np.ascontiguousarray(bq[sl].reshape(HL, 128).T),
                "bk": np.ascontiguousarray(bk[sl].reshape(HL, 128).T),
                "woT3": woT,
                "bo_bc": bo_bc,
                "ones_bf": ones_bf,
            }
        )
    return in_maps


def run(in_maps, trace=False):
    nc = _get_nc()
    return run_bass_kernel_spmd(nc, in_maps, core_ids=list(range(NC)), trace=trace)


def kernel(x, Wq, bq, Wk, bk, Wv, bv, Wo, bo):
    args = [np.asarray(a, dtype=np.float32) for a in (x, Wq, bq, Wk, bk, Wv, bv, Wo, bo)]
    in_maps = _prep_in_maps(*args)
    res = run(in_maps)
    out = np.zeros((B, S, H), dtype=np.float32)
    for c in range(NC):
        o = np.asarray(res.results[c]["out"], dtype=np.float32)
        for b in range(B):
            out[b, c * RPB:(c + 1) * RPB, :] = o[b * RPB:(b + 1) * RPB, :]
    return out
